# revision 2
# baseline (speedup 1.0000x reference)
"""PolyNet kernel for 8 trn2 NeuronCores (data-parallel over batch).

Algebraic structure exploited (all folds done host-side in float64):
  - The g-branch (og1, xg2, xg3) and the layer-3 f output (xf3) only reach
    the final output through the width-1 projection Wfc.  Folding Wfc into
    those weights collapses their [16k, 128] GEMMs into length-16k vectors,
    whose outer-product parts are 128x128 bilinear forms:
        sum_ij o1_i ot_j G[i,j] = sum_j (o1 @ G)_j * ot_j .
  - Only xf2 = xx2 @ Wf2 survives as a real GEMM (its output feeds layer 3's
    outer product), computed on-chip without materializing the outer product
    in DRAM: per 128-feature chunk i, outerT_i[j, b] = o1T[j,b] * o1T[i,b]
    is built with a PE row-broadcast (staircase selector matmul) + one DVE
    tensor-tensor multiply, then immediately consumed by the accumulating
    chunk matmul.

Device layout is feature-on-partition ("transposed"): activations are
[features, batch_free] so the TensorEngine can contract over features.
"""

import sys
import numpy as np

for _p in ("/opt/trn_rl_repo",):
    if _p not in sys.path:
        sys.path.insert(0, _p)

N_CORES = 8
B, D_IN, NF = 8192, 64, 128
R = B // N_CORES          # rows per core
BF = 512                  # batch columns per device tile
TILES = R // BF           # 2
NGRP = 8                  # DMA split groups for the big weight tensors
GRP = NF // NGRP          # chunks per group


def _build_bass():
    import concourse.bacc as bacc
    import concourse.mybir as mybir
    import concourse.tile as tile
    from contextlib import ExitStack

    bf16 = mybir.dt.bfloat16
    f32 = mybir.dt.float32
    AF = mybir.ActivationFunctionType

    nc = bacc.Bacc(
        "TRN2",
        target_bir_lowering=False,
        debug=False,
        enable_asserts=True,
        num_devices=N_CORES,
    )

    dram = {}

    def din(name, shape, dt=bf16):
        dram[name] = nc.dram_tensor(name, shape, dt, kind="ExternalInput").ap()
        return dram[name]

    xT = din("xT", [D_IN, R])
    w1f = din("w1f", [D_IN, NF])
    b1f = din("b1f", [NF, 1], f32)
    a2f = din("a2f", [NF, NF])
    b2f = din("b2f", [NF, 1], f32)
    o2f = din("o2f", [NF, NF * NF])      # chunk-major packed: [j, (i,m)]
    stair = din("stair", [NF, NF * NF])  # stair[p, (i,m)] = 1 if p == i
    g2g = din("g2g", [NF, NF])
    g3f = din("g3f", [NF, NF])
    g3g = din("g3g", [NF, NF])
    ulin = din("ulin", [D_IN, 1])
    vot1 = din("vot1", [NF, 1])
    vxf2 = din("vxf2", [NF, 1])
    out = nc.dram_tensor("out", [TILES, BF], f32, kind="ExternalOutput").ap()

    LOOK = 4  # chunk software-pipeline lookahead

    with tile.TileContext(nc) as tc, ExitStack() as ctx:
        consts = ctx.enter_context(tc.tile_pool(name="consts", bufs=1))
        o1p = ctx.enter_context(tc.tile_pool(name="o1", bufs=2))
        xf2p = ctx.enter_context(tc.tile_pool(name="xf2sb", bufs=2))
        hp = ctx.enter_context(tc.tile_pool(name="hsb", bufs=2))
        bcp = ctx.enter_context(tc.tile_pool(name="bcsb", bufs=LOOK + 2))
        obp = ctx.enter_context(tc.tile_pool(name="obsb", bufs=LOOK + 2))
        pp = ctx.enter_context(tc.tile_pool(name="psb", bufs=2))
        ps_bc = ctx.enter_context(tc.tile_pool(name="ps_bc", bufs=3, space="PSUM"))
        ps_xf2 = ctx.enter_context(tc.tile_pool(name="ps_xf2", bufs=2, space="PSUM"))
        ps_mix = ctx.enter_context(tc.tile_pool(name="ps_mix", bufs=2, space="PSUM"))
        ps_acc = ctx.enter_context(tc.tile_pool(name="ps_acc", bufs=1, space="PSUM"))

        # ---- load constants / weights ----
        xT_sb = consts.tile([D_IN, R], bf16)
        nc.sync.dma_start(xT_sb[:], xT)
        w1f_sb = consts.tile([D_IN, NF], bf16)
        nc.sync.dma_start(w1f_sb[:], w1f)
        b1f_sb = consts.tile([NF, 1], f32)
        nc.sync.dma_start(b1f_sb[:], b1f)
        a2f_sb = consts.tile([NF, NF], bf16)
        nc.sync.dma_start(a2f_sb[:], a2f)
        b2f_sb = consts.tile([NF, 1], f32)
        nc.sync.dma_start(b2f_sb[:], b2f)
        g2g_sb = consts.tile([NF, NF], bf16)
        nc.sync.dma_start(g2g_sb[:], g2g)
        g3f_sb = consts.tile([NF, NF], bf16)
        nc.sync.dma_start(g3f_sb[:], g3f)
        g3g_sb = consts.tile([NF, NF], bf16)
        nc.sync.dma_start(g3g_sb[:], g3g)
        ulin_sb = consts.tile([D_IN, 1], bf16)
        nc.sync.dma_start(ulin_sb[:], ulin)
        vot1_sb = consts.tile([NF, 1], bf16)
        nc.sync.dma_start(vot1_sb[:], vot1)
        vxf2_sb = consts.tile([NF, 1], bf16)
        nc.sync.dma_start(vxf2_sb[:], vxf2)
        ones_sb = consts.tile([NF, 1], bf16)
        nc.vector.memset(ones_sb[:], 1.0)

        # big weights split into NGRP groups so early chunks unblock early
        stair_g = []
        o2f_g = []
        for g in range(NGRP):
            sg = consts.tile([NF, GRP * NF], bf16, tag=f"stair{g}")
            nc.sync.dma_start(sg[:], stair[:, g * GRP * NF:(g + 1) * GRP * NF])
            stair_g.append(sg)
            og = consts.tile([NF, GRP * NF], bf16, tag=f"o2f{g}")
            nc.sync.dma_start(og[:], o2f[:, g * GRP * NF:(g + 1) * GRP * NF])
            o2f_g.append(og)

        for t in range(TILES):
            xt = xT_sb[:, t * BF:(t + 1) * BF]

            # ---- layer 1: o1T = W1f.T @ xT + b1f  (bf16) ----
            ot1_ps = ps_mix.tile([NF, BF], f32, tag="mix")
            nc.tensor.matmul(ot1_ps[:], w1f_sb[:], xt, start=True, stop=True)
            o1 = o1p.tile([NF, BF], bf16)
            nc.scalar.activation(o1[:], ot1_ps[:], AF.Identity,
                                 bias=b1f_sb[:, 0:1], scale=1.0)

            # ---- bilinear h tiles (need only o1) ----
            h_sb = []
            for gw in (g2g_sb, g3f_sb, g3g_sb):
                h_ps = ps_mix.tile([NF, BF], f32, tag="mix")
                nc.tensor.matmul(h_ps[:], gw[:], o1[:], start=True, stop=True)
                h = hp.tile([NF, BF], bf16, tag="hsb")
                nc.scalar.copy(h[:], h_ps[:])
                h_sb.append(h)
            h2g, h3f, h3g = h_sb

            # ---- output accumulator [1, BF] ----
            acc = ps_acc.tile([1, BF], f32)
            nc.tensor.matmul(acc[:], ulin_sb[:], xt, start=True, stop=False)
            nc.tensor.matmul(acc[:], vot1_sb[:], o1[:], start=False, stop=False)
            p2g = pp.tile([NF, BF], bf16, tag="psb")
            nc.vector.tensor_mul(p2g[:], o1[:], h2g[:])
            nc.tensor.matmul(acc[:], ones_sb[:], p2g[:], start=False, stop=False)

            # ---- layer 2 f GEMM with on-chip transposed outer product ----
            xf2_ps = ps_xf2.tile([NF, BF], f32)
            nc.tensor.matmul(xf2_ps[:], a2f_sb[:], o1[:], start=True, stop=False)

            ob_tiles = {}

            def emit_chunk_front(i):
                # PE: broadcast row i of o1 across partitions via staircase
                g, r = divmod(i, GRP)
                bc_ps = ps_bc.tile([NF, BF], f32, tag="bc")
                nc.tensor.matmul(bc_ps[:], stair_g[g][:, r * NF:(r + 1) * NF],
                                 o1[:], start=True, stop=True)
                bc = bcp.tile([NF, BF], bf16, tag="bc_sb")
                nc.scalar.copy(bc[:], bc_ps[:])
                ob = obp.tile([NF, BF], bf16, tag="ob_sb")
                nc.vector.tensor_mul(ob[:], o1[:], bc[:])
                ob_tiles[i] = ob

            def emit_chunk_mm(i):
                g, r = divmod(i, GRP)
                nc.tensor.matmul(xf2_ps[:], o2f_g[g][:, r * NF:(r + 1) * NF],
                                 ob_tiles.pop(i)[:], start=False,
                                 stop=(i == NF - 1))

            for i in range(LOOK):
                emit_chunk_front(i)
            for i in range(NF):
                if i + LOOK < NF:
                    emit_chunk_front(i + LOOK)
                emit_chunk_mm(i)

            xf2 = xf2p.tile([NF, BF], bf16)
            nc.scalar.activation(xf2[:], xf2_ps[:], AF.Identity,
                                 bias=b2f_sb[:, 0:1], scale=1.0)

            # ---- late scalar contributions ----
            nc.tensor.matmul(acc[:], vxf2_sb[:], xf2[:], start=False, stop=False)
            p3f = pp.tile([NF, BF], bf16, tag="psb")
            nc.vector.tensor_mul(p3f[:], xf2[:], h3f[:])
            nc.tensor.matmul(acc[:], ones_sb[:], p3f[:], start=False, stop=False)
            p3g = pp.tile([NF, BF], bf16, tag="psb")
            nc.vector.tensor_mul(p3g[:], xf2[:], h3g[:])
            nc.tensor.matmul(acc[:], ones_sb[:], p3g[:], start=False, stop=True)

            acc_sb = pp.tile([1, BF], f32, tag="acc_sb")
            nc.scalar.copy(acc_sb[:], acc[:])
            nc.sync.dma_start(out[t:t + 1, :], acc_sb[:])

    nc.compile()
    return nc


_CACHE = {}


def _get_nc():
    if "nc" not in _CACHE:
        _CACHE["nc"] = _build_bass()
    return _CACHE["nc"]


def _host_fold(inputs):
    import ml_dtypes

    I = {k: np.asarray(v, np.float64) for k, v in inputs.items()}
    x = I["x"]
    bias0 = I["bias0"]
    Wf1, bf1, Wg1, bg1 = I["Wf1"], I["bf1"], I["Wg1"], I["bg1"]
    Wf2, bf2, Wg2, bg2 = I["Wf2"], I["bf2"], I["Wg2"], I["bg2"]
    Wf3, bf3, Wg3, bg3 = I["Wf3"], I["bf3"], I["Wg3"], I["bg3"]
    Wfc, bfc = I["Wfc"], I["bfc"]

    wfc = Wfc[:, 0]
    v_ot1 = wfc[1:129]
    v_f2 = wfc[129:257]
    v_f3 = wfc[257:385]
    v_g1 = wfc[385:513]
    v_g2 = wfc[513:641]
    v_g3 = wfc[641:769]

    W1f_x = Wf1[1:]
    b1f = bf1 + bias0 * Wf1[0]
    W1g_x = Wg1[1:]
    b1g = bg1 + bias0 * Wg1[0]

    A2f = Wf2[1:129]
    O2f = Wf2[129:]
    b2f = bf2 + bias0 * Wf2[0]

    wg2 = Wg2 @ v_g2
    wf3 = Wf3 @ v_f3
    wg3 = Wg3 @ v_g3
    G2g = wg2[129:].reshape(NF, NF)
    G3f = wf3[257:].reshape(NF, NF)
    G3g = wg3[257:].reshape(NF, NF)

    ulin_v = W1f_x @ v_ot1 + W1g_x @ v_g1
    vot1_v = wg2[1:129] + wf3[1:129] + wg3[1:129]
    vxf2_v = v_f2 + wf3[129:257] + wg3[129:257]
    call = (bfc[0] + bias0 * (wfc[0] + wg2[0] + wf3[0] + wg3[0])
            + b1f @ v_ot1 + b1g @ v_g1 + bg2 @ v_g2 + bf3 @ v_f3 + bg3 @ v_g3)

    bf16 = ml_dtypes.bfloat16

    # o2f packed [j, (i,m)] = O2f[(i,j), m]
    o2f_pack = np.ascontiguousarray(
        O2f.reshape(NF, NF, NF).transpose(1, 0, 2).reshape(NF, NF * NF)
    ).astype(bf16)
    stair3 = np.zeros((NF, NF, NF), dtype=bf16)
    stair3[np.arange(NF), np.arange(NF), :] = bf16(1.0)
    stair = stair3.reshape(NF, NF * NF)

    weights = {
        "w1f": W1f_x.astype(bf16),
        "b1f": b1f.reshape(NF, 1).astype(np.float32),
        "a2f": A2f.astype(bf16),
        "b2f": b2f.reshape(NF, 1).astype(np.float32),
        "o2f": o2f_pack,
        "stair": stair,
        "g2g": G2g.astype(bf16),
        "g3f": G3f.astype(bf16),
        "g3g": G3g.astype(bf16),
        "ulin": ulin_v.reshape(D_IN, 1).astype(bf16),
        "vot1": vot1_v.reshape(NF, 1).astype(bf16),
        "vxf2": vxf2_v.reshape(NF, 1).astype(bf16),
    }
    return weights, call


def kernel(**inputs):
    import ml_dtypes
    from concourse.bass_utils import run_bass_kernel_spmd

    nc = _get_nc()
    weights, call = _host_fold(inputs)

    x = np.asarray(inputs["x"], np.float32)
    bf16 = ml_dtypes.bfloat16
    in_maps = []
    for c in range(N_CORES):
        shard = np.ascontiguousarray(x[c * R:(c + 1) * R].T).astype(bf16)
        m = dict(weights)
        m["xT"] = shard
        in_maps.append(m)

    res = run_bass_kernel_spmd(nc, in_maps, core_ids=list(range(N_CORES)))
    out = np.empty((B, 1), np.float32)
    for c in range(N_CORES):
        out[c * R:(c + 1) * R, 0] = res.results[c]["out"].reshape(R) + np.float32(call)
    return out


# revision 6
# speedup vs baseline: 1.2127x; 1.2127x over previous
"""PolyNet kernel for 8 trn2 NeuronCores (data-parallel over batch).

Algebraic structure exploited (all folds done host-side in float64):
  - The g-branch (og1, xg2, xg3) and the layer-3 f output (xf3) only reach
    the final output through the width-1 projection Wfc.  Folding Wfc into
    those weights collapses their [16k, 128] GEMMs into length-16k vectors,
    whose outer-product parts are 128x128 bilinear forms:
        sum_ij o1_i ot_j G[i,j] = sum_j (o1 @ G)_j * ot_j .
  - Only xf2 = xx2 @ Wf2 survives as a real tensor op (its output feeds
    layer 3's outer product).  Its outer-product part is a bank of 128
    bilinear forms xf2_m = o1^T G_m o1; each symmetrized G_m is
    eigendecomposed on the host:  xf2_m = sum_r s_{m,r} (u_{m,r} . o1)^2.
    On device that is: project z = Vp^T o1 (TensorE), square elementwise
    (single-source op, split across ScalarE/VectorE), then a +-1
    block-diagonal matmul back into PSUM.  No broadcast, no tensor-tensor
    multiply, no PSUM->SBUF copies of wide data.

Device layout is feature-on-partition ("transposed"): activations are
[features, batch_free] so the TensorEngine can contract over features.
"""

import sys
import numpy as np

for _p in ("/opt/trn_rl_repo",):
    if _p not in sys.path:
        sys.path.insert(0, _p)

N_CORES = 8
B, D_IN, NF = 8192, 64, 128
R = B // N_CORES          # rows per core
HB = 512                  # half-batch: free dim of one PSUM-bank unit
NH = R // HB              # 2 halves
LOOK = 2                  # chunk software-pipeline lookahead (in (c,h) units)
ACT_SHARE = 5             # of every 9 squares, this many go to ScalarE


def _square_op():
    """Custom single-stream DVE square: out = in0^2 with ONE tensor read,
    so VectorE can consume f32 PSUM directly (TensorTensor would need two
    PSUM reads, which the ISA forbids)."""
    from concourse import dve_ops
    from concourse.dve_spec import Spec, Src0, sq

    for op in dve_ops.OPS:
        if op.name == "SQUARE_ANT":
            return op
    op = dve_ops.DveOp(
        "SQUARE_ANT",
        Spec(body=sq(Src0),
             reference=lambda in0, in1, s0, s1, imm2: (in0.astype(np.float32) ** 2)),
        subdim=False,
        uops_sha={"v3": "7bd23a2deee7f188", "v4": "80f1201cc018d83b"},
    )
    dve_ops.OPS.append(op)
    dve_ops._SUB_OPCODE_FOR_NAME[op.name] = (
        dve_ops._CUSTOM_DVE_ROW_BASE + len(dve_ops.OPS) - 1
    )
    return op


def _build_bass():
    import concourse.bacc as bacc
    import concourse.mybir as mybir
    import concourse.tile as tile
    from contextlib import ExitStack

    square = _square_op()

    bf16 = mybir.dt.bfloat16
    f32 = mybir.dt.float32
    AF = mybir.ActivationFunctionType

    nc = bacc.Bacc(
        "TRN2",
        target_bir_lowering=False,
        debug=False,
        enable_asserts=True,
        num_devices=N_CORES,
    )

    def din(name, shape, dt=bf16):
        return nc.dram_tensor(name, shape, dt, kind="ExternalInput").ap()

    xT = din("xT", [D_IN, R])
    w1f = din("w1f", [D_IN, NF])
    b1f = din("b1f", [NF, 1], f32)
    a2f = din("a2f", [NF, NF])
    b2f = din("b2f", [NF, 1], f32)
    vp = din("vp", [NF, NF * NF])     # proj chunks: vp[:, c*NF+r] = sqrt|lam| * eigvec
    s2 = din("s2", [NF, NF * NF])     # block-diag sign chunks
    g2g = din("g2g", [NF, NF])
    g3f = din("g3f", [NF, NF])
    g3g = din("g3g", [NF, NF])
    ulin = din("ulin", [D_IN, 1])
    vot1 = din("vot1", [NF, 1])
    vxf2 = din("vxf2", [NF, 1])
    out = nc.dram_tensor("out", [1, R], f32, kind="ExternalOutput").ap()

    NGRP = 8
    GRP = NF // NGRP

    with tile.TileContext(nc) as tc, ExitStack() as ctx:
        consts = ctx.enter_context(tc.tile_pool(name="consts", bufs=1))
        sb1 = ctx.enter_context(tc.tile_pool(name="sb1", bufs=1))
        zsqp = ctx.enter_context(tc.tile_pool(name="zsq", bufs=LOOK + 4))
        ps_z = ctx.enter_context(tc.tile_pool(name="ps_z", bufs=4, space="PSUM"))
        ps_xf2 = ctx.enter_context(tc.tile_pool(name="ps_xf2", bufs=2, space="PSUM"))
        ps_acc = ctx.enter_context(tc.tile_pool(name="ps_acc", bufs=2, space="PSUM"))

        # ---- constants / weights ----
        xT_sb = consts.tile([D_IN, R], bf16)
        nc.sync.dma_start(xT_sb[:], xT)
        w1f_sb = consts.tile([D_IN, NF], bf16)
        nc.sync.dma_start(w1f_sb[:], w1f)
        b1f_sb = consts.tile([NF, 1], f32)
        nc.sync.dma_start(b1f_sb[:], b1f)
        a2f_sb = consts.tile([NF, NF], bf16)
        nc.sync.dma_start(a2f_sb[:], a2f)
        b2f_sb = consts.tile([NF, 1], f32)
        nc.sync.dma_start(b2f_sb[:], b2f)
        g2g_sb = consts.tile([NF, NF], bf16)
        nc.sync.dma_start(g2g_sb[:], g2g)
        g3f_sb = consts.tile([NF, NF], bf16)
        nc.sync.dma_start(g3f_sb[:], g3f)
        g3g_sb = consts.tile([NF, NF], bf16)
        nc.sync.dma_start(g3g_sb[:], g3g)
        ulin_sb = consts.tile([D_IN, 1], bf16)
        nc.sync.dma_start(ulin_sb[:], ulin)
        vot1_sb = consts.tile([NF, 1], bf16)
        nc.sync.dma_start(vot1_sb[:], vot1)
        vxf2_sb = consts.tile([NF, 1], bf16)
        nc.sync.dma_start(vxf2_sb[:], vxf2)
        ones_sb = consts.tile([NF, 1], bf16)
        nc.vector.memset(ones_sb[:], 1.0)

        vp_g, s2_g = [], []
        for g in range(NGRP):
            vg = consts.tile([NF, GRP * NF], bf16, tag=f"vp{g}")
            nc.sync.dma_start(vg[:], vp[:, g * GRP * NF:(g + 1) * GRP * NF])
            vp_g.append(vg)
            sg = consts.tile([NF, GRP * NF], bf16, tag=f"s2{g}")
            nc.sync.dma_start(sg[:], s2[:, g * GRP * NF:(g + 1) * GRP * NF])
            s2_g.append(sg)

        def chunk_ap(tiles, c):
            g, rr = divmod(c, GRP)
            return tiles[g][:, rr * NF:(rr + 1) * NF]

        # ---- layer 1 + bilinear h tiles (all [NF, R] bf16 in SBUF) ----
        o1 = sb1.tile([NF, R], bf16)
        xf2 = sb1.tile([NF, R], bf16)
        h2g = sb1.tile([NF, R], bf16)
        h3f = sb1.tile([NF, R], bf16)
        h3g = sb1.tile([NF, R], bf16)
        p2g = sb1.tile([NF, R], bf16)
        p3f = sb1.tile([NF, R], bf16)
        p3g = sb1.tile([NF, R], bf16)
        acc_sb = sb1.tile([1, R], f32)

        for h in range(NH):
            s = slice(h * HB, (h + 1) * HB)
            ps = ps_z.tile([NF, HB], f32, tag="z", name="zps")
            nc.tensor.matmul(ps[:], w1f_sb[:], xT_sb[:, s], start=True, stop=True)
            nc.scalar.activation(o1[:, s], ps[:], AF.Identity,
                                 bias=b1f_sb[:, 0:1], scale=1.0)
        for gw, ht in ((g2g_sb, h2g), (g3f_sb, h3f), (g3g_sb, h3g)):
            for h in range(NH):
                s = slice(h * HB, (h + 1) * HB)
                ps = ps_z.tile([NF, HB], f32, tag="z", name="zps")
                nc.tensor.matmul(ps[:], gw[:], o1[:, s], start=True, stop=True)
                nc.scalar.copy(ht[:, s], ps[:])

        nc.vector.tensor_mul(p2g[:], o1[:], h2g[:])

        # ---- output accumulators (one per half) ----
        acc = [ps_acc.tile([1, HB], f32, tag="acc", name=f"acc{h}") for h in range(NH)]
        for h in range(NH):
            s = slice(h * HB, (h + 1) * HB)
            nc.tensor.matmul(acc[h][:], ulin_sb[:], xT_sb[:, s], start=True, stop=False)
            nc.tensor.matmul(acc[h][:], vot1_sb[:], o1[:, s], start=False, stop=False)
            nc.tensor.matmul(acc[h][:], ones_sb[:], p2g[:, s], start=False, stop=False)

        # ---- xf2 = ot1 @ A2f + sum_r s (u . o1)^2 + b2f ----
        xf2_ps = [ps_xf2.tile([NF, HB], f32, tag="xf2", name=f"xf2ps{h}") for h in range(NH)]
        for h in range(NH):
            s = slice(h * HB, (h + 1) * HB)
            nc.tensor.matmul(xf2_ps[h][:], a2f_sb[:], o1[:, s], start=True, stop=False)

        NU = NF * NH  # 256 (chunk, half) units
        z_ps = {}
        zsq_sb = {}

        def unit(u):
            return divmod(u, NH)  # (c, h)

        def emit_z(u):
            c, h = unit(u)
            s = slice(h * HB, (h + 1) * HB)
            ps = ps_z.tile([NF, HB], f32, tag="z", name="zps")
            nc.tensor.matmul(ps[:], chunk_ap(vp_g, c), o1[:, s], start=True, stop=True)
            z_ps[u] = ps

        def emit_square(u):
            ps = z_ps.pop(u)
            zq = zsqp.tile([NF, HB], bf16, tag="zsq", name="zsq")
            if u % 9 < ACT_SHARE:
                nc.scalar.square(zq[:], ps[:])
            else:
                nc.vector._custom_dve(square, out=zq[:], in0=ps[:])
            zsq_sb[u] = zq

        def emit_final(u):
            c, h = unit(u)
            nc.tensor.matmul(xf2_ps[h][:], chunk_ap(s2_g, c), zsq_sb.pop(u)[:],
                             start=False, stop=(u >= NU - NH))

        for u in range(LOOK):
            emit_z(u)
            emit_square(u)
        for u in range(NU):
            if u + LOOK < NU:
                emit_z(u + LOOK)
                emit_square(u + LOOK)
            emit_final(u)

        for h in range(NH):
            s = slice(h * HB, (h + 1) * HB)
            nc.scalar.activation(xf2[:, s], xf2_ps[h][:], AF.Identity,
                                 bias=b2f_sb[:, 0:1], scale=1.0)

        # ---- late scalar contributions ----
        nc.vector.tensor_mul(p3f[:], xf2[:], h3f[:])
        nc.vector.tensor_mul(p3g[:], xf2[:], h3g[:])
        for h in range(NH):
            s = slice(h * HB, (h + 1) * HB)
            nc.tensor.matmul(acc[h][:], vxf2_sb[:], xf2[:, s], start=False, stop=False)
            nc.tensor.matmul(acc[h][:], ones_sb[:], p3f[:, s], start=False, stop=False)
            nc.tensor.matmul(acc[h][:], ones_sb[:], p3g[:, s], start=False, stop=True)
            nc.scalar.copy(acc_sb[:, s], acc[h][:])
        nc.sync.dma_start(out, acc_sb[:])

    nc.compile()
    return nc


_CACHE = {}


def _get_nc():
    if "nc" not in _CACHE:
        _CACHE["nc"] = _build_bass()
    return _CACHE["nc"]


def _host_fold(inputs):
    import ml_dtypes

    I = {k: np.asarray(v, np.float64) for k, v in inputs.items()}
    x = I["x"]
    bias0 = I["bias0"]
    Wf1, bf1, Wg1, bg1 = I["Wf1"], I["bf1"], I["Wg1"], I["bg1"]
    Wf2, bf2, Wg2, bg2 = I["Wf2"], I["bf2"], I["Wg2"], I["bg2"]
    Wf3, bf3, Wg3, bg3 = I["Wf3"], I["bf3"], I["Wg3"], I["bg3"]
    Wfc, bfc = I["Wfc"], I["bfc"]

    wfc = Wfc[:, 0]
    v_ot1 = wfc[1:129]
    v_f2 = wfc[129:257]
    v_f3 = wfc[257:385]
    v_g1 = wfc[385:513]
    v_g2 = wfc[513:641]
    v_g3 = wfc[641:769]

    W1f_x = Wf1[1:]
    b1f = bf1 + bias0 * Wf1[0]
    W1g_x = Wg1[1:]
    b1g = bg1 + bias0 * Wg1[0]

    A2f = Wf2[1:129]
    O2f = Wf2[129:]
    b2f = bf2 + bias0 * Wf2[0]

    wg2 = Wg2 @ v_g2
    wf3 = Wf3 @ v_f3
    wg3 = Wg3 @ v_g3
    G2g = wg2[129:].reshape(NF, NF)
    G3f = wf3[257:].reshape(NF, NF)
    G3g = wg3[257:].reshape(NF, NF)

    ulin_v = W1f_x @ v_ot1 + W1g_x @ v_g1
    vot1_v = wg2[1:129] + wf3[1:129] + wg3[1:129]
    vxf2_v = v_f2 + wf3[129:257] + wg3[129:257]
    call = (bfc[0] + bias0 * (wfc[0] + wg2[0] + wf3[0] + wg3[0])
            + b1f @ v_ot1 + b1g @ v_g1 + bg2 @ v_g2 + bf3 @ v_f3 + bg3 @ v_g3)

    # eigendecomposition of the symmetrized bilinear bank
    V = np.empty((NF, NF, NF))      # V[m][k, r] = sqrt|lam| eigvec
    S = np.empty((NF, NF))          # S[m, r] = sign(lam)
    for m in range(NF):
        G = O2f[:, m].reshape(NF, NF)
        lam, vec = np.linalg.eigh((G + G.T) / 2)
        V[m] = vec * np.sqrt(np.abs(lam))[None, :]
        S[m] = np.sign(lam)

    bf16 = ml_dtypes.bfloat16
    vp = np.ascontiguousarray(V.transpose(1, 0, 2).reshape(NF, NF * NF)).astype(bf16)
    s2_3 = np.zeros((NF, NF, NF))
    s2_3[:, np.arange(NF), np.arange(NF)] = S.T
    s2 = np.ascontiguousarray(s2_3.reshape(NF, NF * NF)).astype(bf16)

    weights = {
        "w1f": W1f_x.astype(bf16),
        "b1f": b1f.reshape(NF, 1).astype(np.float32),
        "a2f": A2f.astype(bf16),
        "b2f": b2f.reshape(NF, 1).astype(np.float32),
        "vp": vp,
        "s2": s2,
        "g2g": G2g.astype(bf16),
        "g3f": G3f.astype(bf16),
        "g3g": G3g.astype(bf16),
        "ulin": ulin_v.reshape(D_IN, 1).astype(bf16),
        "vot1": vot1_v.reshape(NF, 1).astype(bf16),
        "vxf2": vxf2_v.reshape(NF, 1).astype(bf16),
    }
    return weights, call


def kernel(**inputs):
    import ml_dtypes
    from concourse.bass_utils import run_bass_kernel_spmd

    nc = _get_nc()
    weights, call = _host_fold(inputs)

    x = np.asarray(inputs["x"], np.float32)
    bf16 = ml_dtypes.bfloat16
    in_maps = []
    for c in range(N_CORES):
        shard = np.ascontiguousarray(x[c * R:(c + 1) * R].T).astype(bf16)
        m = dict(weights)
        m["xT"] = shard
        in_maps.append(m)

    res = run_bass_kernel_spmd(nc, in_maps, core_ids=list(range(N_CORES)))
    out = np.empty((B, 1), np.float32)
    for c in range(N_CORES):
        out[c * R:(c + 1) * R, 0] = res.results[c]["out"].reshape(R) + np.float32(call)
    return out


# revision 8
# speedup vs baseline: 1.2819x; 1.0571x over previous
"""PolyNet kernel for 8 trn2 NeuronCores (data-parallel over batch).

Algebraic structure exploited (all folds done host-side in float64):
  - The g-branch (og1, xg2, xg3) and the layer-3 f output (xf3) only reach
    the final output through the width-1 projection Wfc.  Folding Wfc into
    those weights collapses their [16k, 128] GEMMs into length-16k vectors,
    whose outer-product parts are 128x128 bilinear forms:
        sum_ij o1_i ot_j G[i,j] = sum_j (o1 @ G)_j * ot_j .
  - Only xf2 = xx2 @ Wf2 survives as a real tensor op (its output feeds
    layer 3's outer product).  Its outer-product part is a bank of 128
    bilinear forms xf2_m = o1^T G_m o1; each symmetrized G_m is
    eigendecomposed on the host:  xf2_m = sum_r s_{m,r} (u_{m,r} . o1)^2.
    On device that is: project z = Vp^T o1 (TensorE), square elementwise
    (single-source op, split across ScalarE/VectorE), then a +-1
    block-diagonal matmul back into PSUM.  No broadcast, no tensor-tensor
    multiply, no PSUM->SBUF copies of wide data.

Device layout is feature-on-partition ("transposed"): activations are
[features, batch_free] so the TensorEngine can contract over features.
"""

import sys
import numpy as np

for _p in ("/opt/trn_rl_repo",):
    if _p not in sys.path:
        sys.path.insert(0, _p)

N_CORES = 8
B, D_IN, NF = 8192, 64, 128
R = B // N_CORES          # rows per core
HB = 512                  # half-batch: free dim of one PSUM-bank unit
NH = R // HB              # 2 halves
LOOK = 3                  # chunk software-pipeline lookahead (in (c,h) units)
ACT_SHARE = 5             # of every 9 squares, this many go to ScalarE


def _square_op():
    """Custom single-stream DVE square: out = in0^2 with ONE tensor read,
    so VectorE can consume f32 PSUM directly (TensorTensor would need two
    PSUM reads, which the ISA forbids)."""
    from concourse import dve_ops
    from concourse.dve_spec import Spec, Src0, sq

    for op in dve_ops.OPS:
        if op.name == "SQUARE_ANT":
            return op
    op = dve_ops.DveOp(
        "SQUARE_ANT",
        Spec(body=sq(Src0),
             reference=lambda in0, in1, s0, s1, imm2: (in0.astype(np.float32) ** 2)),
        subdim=False,
        uops_sha={"v3": "7bd23a2deee7f188", "v4": "80f1201cc018d83b"},
    )
    dve_ops.OPS.append(op)
    dve_ops._SUB_OPCODE_FOR_NAME[op.name] = (
        dve_ops._CUSTOM_DVE_ROW_BASE + len(dve_ops.OPS) - 1
    )
    return op


def _build_bass():
    import concourse.bacc as bacc
    import concourse.mybir as mybir
    import concourse.tile as tile
    from contextlib import ExitStack

    square = _square_op()

    bf16 = mybir.dt.bfloat16
    f32 = mybir.dt.float32
    AF = mybir.ActivationFunctionType

    nc = bacc.Bacc(
        "TRN2",
        target_bir_lowering=False,
        debug=False,
        enable_asserts=True,
        num_devices=N_CORES,
    )

    def din(name, shape, dt=bf16):
        return nc.dram_tensor(name, shape, dt, kind="ExternalInput").ap()

    xT = din("xT", [D_IN, R])
    w1f = din("w1f", [D_IN, NF])
    b1f = din("b1f", [NF, 1], f32)
    a2f = din("a2f", [NF, NF])
    b2f = din("b2f", [NF, 1], f32)
    vp = din("vp", [NF, NF * NF])     # proj chunks: vp[:, c*NF+r] = sqrt|lam| * eigvec
    s2 = din("s2", [NF, NF * NF])     # block-diag sign chunks
    g2g = din("g2g", [NF, NF])
    g3f = din("g3f", [NF, NF])
    g3g = din("g3g", [NF, NF])
    ulin = din("ulin", [D_IN, 1])
    vot1 = din("vot1", [NF, 1])
    vxf2 = din("vxf2", [NF, 1])
    out = nc.dram_tensor("out", [1, R], f32, kind="ExternalOutput").ap()

    NGRP = 16
    GRP = NF // NGRP

    with tile.TileContext(nc) as tc, ExitStack() as ctx:
        consts = ctx.enter_context(tc.tile_pool(name="consts", bufs=1))
        sb1 = ctx.enter_context(tc.tile_pool(name="sb1", bufs=1))
        zsqp = ctx.enter_context(tc.tile_pool(name="zsq", bufs=LOOK + 4))
        ps_z = ctx.enter_context(tc.tile_pool(name="ps_z", bufs=5, space="PSUM"))
        ps_xf2 = ctx.enter_context(tc.tile_pool(name="ps_xf2", bufs=2, space="PSUM"))
        ps_acc = ctx.enter_context(tc.tile_pool(name="ps_acc", bufs=1, space="PSUM"))

        # ---- constants / weights ----
        xT_sb = consts.tile([D_IN, R], bf16)
        nc.sync.dma_start(xT_sb[:], xT)
        w1f_sb = consts.tile([D_IN, NF], bf16)
        nc.sync.dma_start(w1f_sb[:], w1f)
        b1f_sb = consts.tile([NF, 1], f32)
        nc.sync.dma_start(b1f_sb[:], b1f)
        a2f_sb = consts.tile([NF, NF], bf16)
        nc.sync.dma_start(a2f_sb[:], a2f)
        b2f_sb = consts.tile([NF, 1], f32)
        nc.sync.dma_start(b2f_sb[:], b2f)
        g2g_sb = consts.tile([NF, NF], bf16)
        nc.sync.dma_start(g2g_sb[:], g2g)
        g3f_sb = consts.tile([NF, NF], bf16)
        nc.sync.dma_start(g3f_sb[:], g3f)
        g3g_sb = consts.tile([NF, NF], bf16)
        nc.sync.dma_start(g3g_sb[:], g3g)
        ulin_sb = consts.tile([D_IN, 1], bf16)
        nc.sync.dma_start(ulin_sb[:], ulin)
        vot1_sb = consts.tile([NF, 1], bf16)
        nc.sync.dma_start(vot1_sb[:], vot1)
        vxf2_sb = consts.tile([NF, 1], bf16)
        nc.sync.dma_start(vxf2_sb[:], vxf2)
        ones_sb = consts.tile([NF, 1], bf16)
        nc.vector.memset(ones_sb[:], 1.0)

        vp_g, s2_g = [], []
        for g in range(NGRP):
            vg = consts.tile([NF, GRP * NF], bf16, tag=f"vp{g}")
            nc.sync.dma_start(vg[:], vp[:, g * GRP * NF:(g + 1) * GRP * NF])
            vp_g.append(vg)
            sg = consts.tile([NF, GRP * NF], bf16, tag=f"s2{g}")
            nc.sync.dma_start(sg[:], s2[:, g * GRP * NF:(g + 1) * GRP * NF])
            s2_g.append(sg)

        def chunk_ap(tiles, c):
            g, rr = divmod(c, GRP)
            return tiles[g][:, rr * NF:(rr + 1) * NF]

        # ---- layer 1 + bilinear h tiles (all [NF, R] bf16 in SBUF) ----
        o1 = sb1.tile([NF, R], bf16)
        xf2 = sb1.tile([NF, R], bf16)
        h2g = sb1.tile([NF, R], bf16)
        h3f = sb1.tile([NF, R], bf16)
        h3g = sb1.tile([NF, R], bf16)
        p2g = sb1.tile([NF, R], bf16)
        p3f = sb1.tile([NF, R], bf16)
        p3g = sb1.tile([NF, R], bf16)
        acc_sb = sb1.tile([1, R], f32)

        for h in range(NH):
            s = slice(h * HB, (h + 1) * HB)
            ps = ps_z.tile([NF, HB], f32, tag="z", name="zps")
            nc.tensor.matmul(ps[:], w1f_sb[:], xT_sb[:, s], start=True, stop=True)
            nc.scalar.activation(o1[:, s], ps[:], AF.Identity,
                                 bias=b1f_sb[:, 0:1], scale=1.0)
        for gw, ht in ((g2g_sb, h2g), (g3f_sb, h3f), (g3g_sb, h3g)):
            for h in range(NH):
                s = slice(h * HB, (h + 1) * HB)
                ps = ps_z.tile([NF, HB], f32, tag="z", name="zps")
                nc.tensor.matmul(ps[:], gw[:], o1[:, s], start=True, stop=True)
                nc.scalar.copy(ht[:, s], ps[:])

        nc.vector.tensor_mul(p2g[:], o1[:], h2g[:])

        # ---- output accumulators (one per half) ----
        acc2 = ps_acc.tile([33, HB], f32, tag="acc", name="acc2")
        acc = [acc2[32 * h:32 * h + 1, :] for h in range(NH)]
        for h in range(NH):
            s = slice(h * HB, (h + 1) * HB)
            nc.tensor.matmul(acc[h][:], ulin_sb[:], xT_sb[:, s], start=True, stop=False)
            nc.tensor.matmul(acc[h][:], vot1_sb[:], o1[:, s], start=False, stop=False)
            nc.tensor.matmul(acc[h][:], ones_sb[:], p2g[:, s], start=False, stop=False)

        # ---- xf2 = ot1 @ A2f + sum_r s (u . o1)^2 + b2f ----
        xf2_ps = [ps_xf2.tile([NF, HB], f32, tag="xf2", name=f"xf2ps{h}") for h in range(NH)]
        for h in range(NH):
            s = slice(h * HB, (h + 1) * HB)
            nc.tensor.matmul(xf2_ps[h][:], a2f_sb[:], o1[:, s], start=True, stop=False)

        NU = NF * NH  # 256 (chunk, half) units
        z_ps = {}
        zsq_sb = {}

        def unit(u):
            return divmod(u, NH)  # (c, h)

        def emit_z(u):
            c, h = unit(u)
            s = slice(h * HB, (h + 1) * HB)
            ps = ps_z.tile([NF, HB], f32, tag="z", name="zps")
            nc.tensor.matmul(ps[:], chunk_ap(vp_g, c), o1[:, s], start=True, stop=True)
            z_ps[u] = ps

        def emit_square(u):
            ps = z_ps.pop(u)
            zq = zsqp.tile([NF, HB], bf16, tag="zsq", name="zsq")
            if (u % 9) % 2 == 0:  # A,D,A,D,A,D,A,D,A -> 5:4 interleaved
                nc.scalar.square(zq[:], ps[:])
            else:
                nc.vector._custom_dve(square, out=zq[:], in0=ps[:])
            zsq_sb[u] = zq

        def emit_final(u):
            c, h = unit(u)
            nc.tensor.matmul(xf2_ps[h][:], chunk_ap(s2_g, c), zsq_sb.pop(u)[:],
                             start=False, stop=(u >= NU - NH))

        for u in range(LOOK):
            emit_z(u)
            emit_square(u)
        for u in range(NU):
            if u + LOOK < NU:
                emit_z(u + LOOK)
                emit_square(u + LOOK)
            emit_final(u)

        for h in range(NH):
            s = slice(h * HB, (h + 1) * HB)
            nc.scalar.activation(xf2[:, s], xf2_ps[h][:], AF.Identity,
                                 bias=b2f_sb[:, 0:1], scale=1.0)

        # ---- late scalar contributions ----
        nc.vector.tensor_mul(p3f[:], xf2[:], h3f[:])
        nc.vector.tensor_mul(p3g[:], xf2[:], h3g[:])
        for h in range(NH):
            s = slice(h * HB, (h + 1) * HB)
            nc.tensor.matmul(acc[h][:], vxf2_sb[:], xf2[:, s], start=False, stop=False)
            nc.tensor.matmul(acc[h][:], ones_sb[:], p3f[:, s], start=False, stop=False)
            nc.tensor.matmul(acc[h][:], ones_sb[:], p3g[:, s], start=False, stop=True)
            nc.scalar.copy(acc_sb[:, s], acc[h][:])  # [1,HB] from partition 32h
        nc.sync.dma_start(out, acc_sb[:])

    nc.compile()
    return nc


_CACHE = {}


def _get_nc():
    if "nc" not in _CACHE:
        _CACHE["nc"] = _build_bass()
    return _CACHE["nc"]


def _host_fold(inputs):
    import ml_dtypes

    I = {k: np.asarray(v, np.float64) for k, v in inputs.items()}
    x = I["x"]
    bias0 = I["bias0"]
    Wf1, bf1, Wg1, bg1 = I["Wf1"], I["bf1"], I["Wg1"], I["bg1"]
    Wf2, bf2, Wg2, bg2 = I["Wf2"], I["bf2"], I["Wg2"], I["bg2"]
    Wf3, bf3, Wg3, bg3 = I["Wf3"], I["bf3"], I["Wg3"], I["bg3"]
    Wfc, bfc = I["Wfc"], I["bfc"]

    wfc = Wfc[:, 0]
    v_ot1 = wfc[1:129]
    v_f2 = wfc[129:257]
    v_f3 = wfc[257:385]
    v_g1 = wfc[385:513]
    v_g2 = wfc[513:641]
    v_g3 = wfc[641:769]

    W1f_x = Wf1[1:]
    b1f = bf1 + bias0 * Wf1[0]
    W1g_x = Wg1[1:]
    b1g = bg1 + bias0 * Wg1[0]

    A2f = Wf2[1:129]
    O2f = Wf2[129:]
    b2f = bf2 + bias0 * Wf2[0]

    wg2 = Wg2 @ v_g2
    wf3 = Wf3 @ v_f3
    wg3 = Wg3 @ v_g3
    G2g = wg2[129:].reshape(NF, NF)
    G3f = wf3[257:].reshape(NF, NF)
    G3g = wg3[257:].reshape(NF, NF)

    ulin_v = W1f_x @ v_ot1 + W1g_x @ v_g1
    vot1_v = wg2[1:129] + wf3[1:129] + wg3[1:129]
    vxf2_v = v_f2 + wf3[129:257] + wg3[129:257]
    call = (bfc[0] + bias0 * (wfc[0] + wg2[0] + wf3[0] + wg3[0])
            + b1f @ v_ot1 + b1g @ v_g1 + bg2 @ v_g2 + bf3 @ v_f3 + bg3 @ v_g3)

    # eigendecomposition of the symmetrized bilinear bank
    V = np.empty((NF, NF, NF))      # V[m][k, r] = sqrt|lam| eigvec
    S = np.empty((NF, NF))          # S[m, r] = sign(lam)
    for m in range(NF):
        G = O2f[:, m].reshape(NF, NF)
        lam, vec = np.linalg.eigh((G + G.T) / 2)
        V[m] = vec * np.sqrt(np.abs(lam))[None, :]
        S[m] = np.sign(lam)

    bf16 = ml_dtypes.bfloat16
    vp = np.ascontiguousarray(V.transpose(1, 0, 2).reshape(NF, NF * NF)).astype(bf16)
    s2_3 = np.zeros((NF, NF, NF))
    s2_3[:, np.arange(NF), np.arange(NF)] = S.T
    s2 = np.ascontiguousarray(s2_3.reshape(NF, NF * NF)).astype(bf16)

    weights = {
        "w1f": W1f_x.astype(bf16),
        "b1f": b1f.reshape(NF, 1).astype(np.float32),
        "a2f": A2f.astype(bf16),
        "b2f": b2f.reshape(NF, 1).astype(np.float32),
        "vp": vp,
        "s2": s2,
        "g2g": G2g.astype(bf16),
        "g3f": G3f.astype(bf16),
        "g3g": G3g.astype(bf16),
        "ulin": ulin_v.reshape(D_IN, 1).astype(bf16),
        "vot1": vot1_v.reshape(NF, 1).astype(bf16),
        "vxf2": vxf2_v.reshape(NF, 1).astype(bf16),
    }
    return weights, call


def kernel(**inputs):
    import ml_dtypes
    from concourse.bass_utils import run_bass_kernel_spmd

    nc = _get_nc()
    weights, call = _host_fold(inputs)

    x = np.asarray(inputs["x"], np.float32)
    bf16 = ml_dtypes.bfloat16
    in_maps = []
    for c in range(N_CORES):
        shard = np.ascontiguousarray(x[c * R:(c + 1) * R].T).astype(bf16)
        m = dict(weights)
        m["xT"] = shard
        in_maps.append(m)

    res = run_bass_kernel_spmd(nc, in_maps, core_ids=list(range(N_CORES)))
    out = np.empty((B, 1), np.float32)
    for c in range(N_CORES):
        out[c * R:(c + 1) * R, 0] = res.results[c]["out"].reshape(R) + np.float32(call)
    return out


# revision 9
# speedup vs baseline: 2.0932x; 1.6328x over previous
"""PolyNet kernel for 8 trn2 NeuronCores (data-parallel over batch).

Algebraic structure exploited (all folds done host-side in float64):
  - The g-branch (og1, xg2, xg3) and the layer-3 f output (xf3) only reach
    the final output through the width-1 projection Wfc.  Folding Wfc into
    those weights collapses their [16k, 128] GEMMs into length-16k vectors,
    whose outer-product parts are 128x128 bilinear forms:
        sum_ij o1_i ot_j G[i,j] = sum_j (o1 @ G)_j * ot_j .
  - Only xf2 = xx2 @ Wf2 survives as a real tensor op (its output feeds
    layer 3's outer product).  Its outer-product part is a bank of 128
    bilinear forms xf2_m = o1^T G_m o1; each symmetrized G_m is
    eigendecomposed on the host:  xf2_m = sum_r s_{m,r} (u_{m,r} . o1)^2.
    On device that is: project z = Vp^T o1 (TensorE), square elementwise
    (single-source op, split across ScalarE/VectorE), then a +-1
    block-diagonal matmul back into PSUM.  No broadcast, no tensor-tensor
    multiply, no PSUM->SBUF copies of wide data.

Device layout is feature-on-partition ("transposed"): activations are
[features, batch_free] so the TensorEngine can contract over features.
"""

import sys
import numpy as np

for _p in ("/opt/trn_rl_repo",):
    if _p not in sys.path:
        sys.path.insert(0, _p)

N_CORES = 8
B, D_IN, NF = 8192, 64, 128
R = B // N_CORES          # rows per core
HB = 512                  # half-batch: free dim of one PSUM-bank unit
NH = R // HB              # 2 halves
NCH = 65                  # projection chunks: 8128 pair-sums + 128 diagonals, padded
NPAD = NCH * NF           # 8320 projections incl. 64 zero-pad
LOOK = 3                  # chunk software-pipeline lookahead (in (c,h) units)
ACT_SHARE = 5             # of every 9 squares, this many go to ScalarE


def _square_op():
    """Custom single-stream DVE square: out = in0^2 with ONE tensor read,
    so VectorE can consume f32 PSUM directly (TensorTensor would need two
    PSUM reads, which the ISA forbids)."""
    from concourse import dve_ops
    from concourse.dve_spec import Spec, Src0, sq

    for op in dve_ops.OPS:
        if op.name == "SQUARE_ANT":
            return op
    op = dve_ops.DveOp(
        "SQUARE_ANT",
        Spec(body=sq(Src0),
             reference=lambda in0, in1, s0, s1, imm2: (in0.astype(np.float32) ** 2)),
        subdim=False,
        uops_sha={"v3": "7bd23a2deee7f188", "v4": "80f1201cc018d83b"},
    )
    dve_ops.OPS.append(op)
    dve_ops._SUB_OPCODE_FOR_NAME[op.name] = (
        dve_ops._CUSTOM_DVE_ROW_BASE + len(dve_ops.OPS) - 1
    )
    return op


def _build_bass():
    import concourse.bacc as bacc
    import concourse.mybir as mybir
    import concourse.tile as tile
    from contextlib import ExitStack

    square = _square_op()

    bf16 = mybir.dt.bfloat16
    f32 = mybir.dt.float32
    AF = mybir.ActivationFunctionType

    nc = bacc.Bacc(
        "TRN2",
        target_bir_lowering=False,
        debug=False,
        enable_asserts=True,
        num_devices=N_CORES,
    )

    def din(name, shape, dt=bf16):
        return nc.dram_tensor(name, shape, dt, kind="ExternalInput").ap()

    xT = din("xT", [D_IN, R])
    w1f = din("w1f", [D_IN, NF])
    b1f = din("b1f", [NF, 1], f32)
    a2f = din("a2f", [NF, NF])
    b2f = din("b2f", [NF, 1], f32)
    vp = din("vp", [NF, NPAD])        # 0/1 pair-indicator projection columns
    s2 = din("s2", [NF, NPAD])        # chunk-major packed polarization coefficients
    g2g = din("g2g", [NF, NF])
    g3f = din("g3f", [NF, NF])
    g3g = din("g3g", [NF, NF])
    ulin = din("ulin", [D_IN, 1])
    vot1 = din("vot1", [NF, 1])
    vxf2 = din("vxf2", [NF, 1])
    out = nc.dram_tensor("out", [1, R], f32, kind="ExternalOutput").ap()

    NGRP = 13
    GRP = NCH // NGRP     # 5 chunks per DMA group

    with tile.TileContext(nc) as tc, ExitStack() as ctx:
        consts = ctx.enter_context(tc.tile_pool(name="consts", bufs=1))
        sb1 = ctx.enter_context(tc.tile_pool(name="sb1", bufs=1))
        zsqp = ctx.enter_context(tc.tile_pool(name="zsq", bufs=LOOK + 4))
        ps_z = ctx.enter_context(tc.tile_pool(name="ps_z", bufs=5, space="PSUM"))
        ps_xf2 = ctx.enter_context(tc.tile_pool(name="ps_xf2", bufs=2, space="PSUM"))
        ps_acc = ctx.enter_context(tc.tile_pool(name="ps_acc", bufs=1, space="PSUM"))

        # ---- constants / weights ----
        xT_sb = consts.tile([D_IN, R], bf16)
        nc.sync.dma_start(xT_sb[:], xT)
        w1f_sb = consts.tile([D_IN, NF], bf16)
        nc.sync.dma_start(w1f_sb[:], w1f)
        b1f_sb = consts.tile([NF, 1], f32)
        nc.sync.dma_start(b1f_sb[:], b1f)
        a2f_sb = consts.tile([NF, NF], bf16)
        nc.sync.dma_start(a2f_sb[:], a2f)
        b2f_sb = consts.tile([NF, 1], f32)
        nc.sync.dma_start(b2f_sb[:], b2f)
        g2g_sb = consts.tile([NF, NF], bf16)
        nc.sync.dma_start(g2g_sb[:], g2g)
        g3f_sb = consts.tile([NF, NF], bf16)
        nc.sync.dma_start(g3f_sb[:], g3f)
        g3g_sb = consts.tile([NF, NF], bf16)
        nc.sync.dma_start(g3g_sb[:], g3g)
        ulin_sb = consts.tile([D_IN, 1], bf16)
        nc.sync.dma_start(ulin_sb[:], ulin)
        vot1_sb = consts.tile([NF, 1], bf16)
        nc.sync.dma_start(vot1_sb[:], vot1)
        vxf2_sb = consts.tile([NF, 1], bf16)
        nc.sync.dma_start(vxf2_sb[:], vxf2)
        ones_sb = consts.tile([NF, 1], bf16)
        nc.vector.memset(ones_sb[:], 1.0)

        vp_g, s2_g = [], []
        for g in range(NGRP):
            vg = consts.tile([NF, GRP * NF], bf16, tag=f"vp{g}")
            nc.sync.dma_start(vg[:], vp[:, g * GRP * NF:(g + 1) * GRP * NF])
            vp_g.append(vg)
            sg = consts.tile([NF, GRP * NF], bf16, tag=f"s2{g}")
            nc.sync.dma_start(sg[:], s2[:, g * GRP * NF:(g + 1) * GRP * NF])
            s2_g.append(sg)

        def chunk_ap(tiles, c):
            g, rr = divmod(c, GRP)
            return tiles[g][:, rr * NF:(rr + 1) * NF]

        # ---- layer 1 + bilinear h tiles (all [NF, R] bf16 in SBUF) ----
        o1 = sb1.tile([NF, R], bf16)
        xf2 = sb1.tile([NF, R], bf16)
        h2g = sb1.tile([NF, R], bf16)
        h3f = sb1.tile([NF, R], bf16)
        h3g = sb1.tile([NF, R], bf16)
        p2g = sb1.tile([NF, R], bf16)
        p3f = sb1.tile([NF, R], bf16)
        p3g = sb1.tile([NF, R], bf16)
        acc_sb = sb1.tile([1, R], f32)

        for h in range(NH):
            s = slice(h * HB, (h + 1) * HB)
            ps = ps_z.tile([NF, HB], f32, tag="z", name="zps")
            nc.tensor.matmul(ps[:], w1f_sb[:], xT_sb[:, s], start=True, stop=True)
            nc.scalar.activation(o1[:, s], ps[:], AF.Identity,
                                 bias=b1f_sb[:, 0:1], scale=1.0)
        for gw, ht in ((g2g_sb, h2g), (g3f_sb, h3f), (g3g_sb, h3g)):
            for h in range(NH):
                s = slice(h * HB, (h + 1) * HB)
                ps = ps_z.tile([NF, HB], f32, tag="z", name="zps")
                nc.tensor.matmul(ps[:], gw[:], o1[:, s], start=True, stop=True)
                nc.scalar.copy(ht[:, s], ps[:])

        nc.vector.tensor_mul(p2g[:], o1[:], h2g[:])

        # ---- output accumulators (one per half) ----
        acc2 = ps_acc.tile([33, HB], f32, tag="acc", name="acc2")
        acc = [acc2[32 * h:32 * h + 1, :] for h in range(NH)]
        for h in range(NH):
            s = slice(h * HB, (h + 1) * HB)
            nc.tensor.matmul(acc[h][:], ulin_sb[:], xT_sb[:, s], start=True, stop=False)
            nc.tensor.matmul(acc[h][:], vot1_sb[:], o1[:, s], start=False, stop=False)
            nc.tensor.matmul(acc[h][:], ones_sb[:], p2g[:, s], start=False, stop=False)

        # ---- xf2 = ot1 @ A2f + sum_r s (u . o1)^2 + b2f ----
        xf2_ps = [ps_xf2.tile([NF, HB], f32, tag="xf2", name=f"xf2ps{h}") for h in range(NH)]
        for h in range(NH):
            s = slice(h * HB, (h + 1) * HB)
            nc.tensor.matmul(xf2_ps[h][:], a2f_sb[:], o1[:, s], start=True, stop=False)

        NU = NCH * NH  # 130 (chunk, half) units
        z_ps = {}
        zsq_sb = {}

        def unit(u):
            return divmod(u, NH)  # (c, h)

        def emit_z(u):
            c, h = unit(u)
            s = slice(h * HB, (h + 1) * HB)
            ps = ps_z.tile([NF, HB], f32, tag="z", name="zps")
            nc.tensor.matmul(ps[:], chunk_ap(vp_g, c), o1[:, s], start=True, stop=True)
            z_ps[u] = ps

        def emit_square(u):
            ps = z_ps.pop(u)
            zq = zsqp.tile([NF, HB], bf16, tag="zsq", name="zsq")
            if (u % 9) % 2 == 0:  # A,D,A,D,A,D,A,D,A -> 5:4 interleaved
                nc.scalar.square(zq[:], ps[:])
            else:
                nc.vector._custom_dve(square, out=zq[:], in0=ps[:])
            zsq_sb[u] = zq

        def emit_final(u):
            c, h = unit(u)
            nc.tensor.matmul(xf2_ps[h][:], chunk_ap(s2_g, c), zsq_sb.pop(u)[:],
                             start=False, stop=(u >= NU - NH))

        for u in range(LOOK):
            emit_z(u)
            emit_square(u)
        for u in range(NU):
            if u + LOOK < NU:
                emit_z(u + LOOK)
                emit_square(u + LOOK)
            emit_final(u)

        for h in range(NH):
            s = slice(h * HB, (h + 1) * HB)
            nc.scalar.activation(xf2[:, s], xf2_ps[h][:], AF.Identity,
                                 bias=b2f_sb[:, 0:1], scale=1.0)

        # ---- late scalar contributions ----
        nc.vector.tensor_mul(p3f[:], xf2[:], h3f[:])
        nc.vector.tensor_mul(p3g[:], xf2[:], h3g[:])
        for h in range(NH):
            s = slice(h * HB, (h + 1) * HB)
            nc.tensor.matmul(acc[h][:], vxf2_sb[:], xf2[:, s], start=False, stop=False)
            nc.tensor.matmul(acc[h][:], ones_sb[:], p3f[:, s], start=False, stop=False)
            nc.tensor.matmul(acc[h][:], ones_sb[:], p3g[:, s], start=False, stop=True)
            nc.scalar.copy(acc_sb[:, s], acc[h][:])  # [1,HB] from partition 32h
        nc.sync.dma_start(out, acc_sb[:])

    nc.compile()
    return nc


_CACHE = {}


def _get_nc():
    if "nc" not in _CACHE:
        _CACHE["nc"] = _build_bass()
    return _CACHE["nc"]


def _host_fold(inputs):
    import ml_dtypes

    I = {k: np.asarray(v, np.float64) for k, v in inputs.items()}
    x = I["x"]
    bias0 = I["bias0"]
    Wf1, bf1, Wg1, bg1 = I["Wf1"], I["bf1"], I["Wg1"], I["bg1"]
    Wf2, bf2, Wg2, bg2 = I["Wf2"], I["bf2"], I["Wg2"], I["bg2"]
    Wf3, bf3, Wg3, bg3 = I["Wf3"], I["bf3"], I["Wg3"], I["bg3"]
    Wfc, bfc = I["Wfc"], I["bfc"]

    wfc = Wfc[:, 0]
    v_ot1 = wfc[1:129]
    v_f2 = wfc[129:257]
    v_f3 = wfc[257:385]
    v_g1 = wfc[385:513]
    v_g2 = wfc[513:641]
    v_g3 = wfc[641:769]

    W1f_x = Wf1[1:]
    b1f = bf1 + bias0 * Wf1[0]
    W1g_x = Wg1[1:]
    b1g = bg1 + bias0 * Wg1[0]

    A2f = Wf2[1:129]
    O2f = Wf2[129:]
    b2f = bf2 + bias0 * Wf2[0]

    wg2 = Wg2 @ v_g2
    wf3 = Wf3 @ v_f3
    wg3 = Wg3 @ v_g3
    G2g = wg2[129:].reshape(NF, NF)
    G3f = wf3[257:].reshape(NF, NF)
    G3g = wg3[257:].reshape(NF, NF)

    ulin_v = W1f_x @ v_ot1 + W1g_x @ v_g1
    vot1_v = wg2[1:129] + wf3[1:129] + wg3[1:129]
    vxf2_v = v_f2 + wf3[129:257] + wg3[129:257]
    call = (bfc[0] + bias0 * (wfc[0] + wg2[0] + wf3[0] + wg3[0])
            + b1f @ v_ot1 + b1g @ v_g1 + bg2 @ v_g2 + bf3 @ v_f3 + bg3 @ v_g3)

    # polarization basis: o1_i o1_j = ((o1_i+o1_j)^2 - o1_i^2 - o1_j^2)/2
    Gt = O2f.reshape(NF, NF, NF)            # [i, j, m]
    A3 = Gt + Gt.transpose(1, 0, 2)         # symmetrized
    iu, ju = np.triu_indices(NF, k=1)       # 8128 pairs i<j
    c_pair = A3[iu, ju, :] / 2.0            # [8128, m]
    rowsum = A3.sum(axis=1)                 # [i, m]
    d_diag = (np.einsum('iim->im', Gt)
              - 0.5 * (rowsum - A3[np.arange(NF), np.arange(NF), :]))

    bf16 = ml_dtypes.bfloat16
    vp_f = np.zeros((NF, NPAD), dtype=bf16)
    pidx = np.arange(len(iu))
    vp_f[iu, pidx] = bf16(1.0)
    vp_f[ju, pidx] = bf16(1.0)
    vp_f[np.arange(NF), 8128 + np.arange(NF)] = bf16(1.0)
    vp = vp_f
    c_full = np.zeros((NPAD, NF))
    c_full[:8128] = c_pair
    c_full[8128:8256] = d_diag
    s2 = np.ascontiguousarray(
        c_full.reshape(NCH, NF, NF).transpose(1, 0, 2).reshape(NF, NPAD)
    ).astype(bf16)

    weights = {
        "w1f": W1f_x.astype(bf16),
        "b1f": b1f.reshape(NF, 1).astype(np.float32),
        "a2f": A2f.astype(bf16),
        "b2f": b2f.reshape(NF, 1).astype(np.float32),
        "vp": vp,
        "s2": s2,
        "g2g": G2g.astype(bf16),
        "g3f": G3f.astype(bf16),
        "g3g": G3g.astype(bf16),
        "ulin": ulin_v.reshape(D_IN, 1).astype(bf16),
        "vot1": vot1_v.reshape(NF, 1).astype(bf16),
        "vxf2": vxf2_v.reshape(NF, 1).astype(bf16),
    }
    return weights, call


def kernel(**inputs):
    import ml_dtypes
    from concourse.bass_utils import run_bass_kernel_spmd

    nc = _get_nc()
    weights, call = _host_fold(inputs)

    x = np.asarray(inputs["x"], np.float32)
    bf16 = ml_dtypes.bfloat16
    in_maps = []
    for c in range(N_CORES):
        shard = np.ascontiguousarray(x[c * R:(c + 1) * R].T).astype(bf16)
        m = dict(weights)
        m["xT"] = shard
        in_maps.append(m)

    res = run_bass_kernel_spmd(nc, in_maps, core_ids=list(range(N_CORES)))
    out = np.empty((B, 1), np.float32)
    for c in range(N_CORES):
        out[c * R:(c + 1) * R, 0] = res.results[c]["out"].reshape(R) + np.float32(call)
    return out
